# revision 1
# baseline (speedup 1.0000x reference)
"""Trainium2 Bass kernel for fused MHA with q/k std-normalization.

Reference computation (per batch b, head h):
    q,k,v = x[b].T @ Wq/Wk/Wv          [T, 64] each
    q = (q - mean_e) / (std_e(ddof=1) + 1e-5)   (same for k)
    attn = softmax(q @ k.T / 8)
    out[b, h*64:(h+1)*64, :] = (attn @ v).T

Sharding: 8 cores = 4 batches x 2 half-head-groups. Core c handles batch
c//2, heads (c%2)*8 .. (c%2)*8+8. Fully head-independent, no collectives.

Per-core pipeline (all shapes f32):
  Phase 1 (QKV projection, all 8 local heads share every x-slice weight load):
    for each t-tile (16 x 128): lhsT = x[d-chunk, t-tile], rhs = packed W
    -> psum [t128, q|k|v per head]; bn_stats/bn_aggr -> mean/var; sigma =
    sqrt(var*64/63); inv = 1/(sigma+1e-5); qhat/khat = (x-m)*inv staged as
    a [t128, qhat|khat 128] tile; PE-transpose -> qkT[h] [e 0:64 = qT,
    64:128 = kT, T]; v -> vtile [s128, 64 | ones] (ones col feeds the
    softmax denominator through the attention*V matmul).
  Phase 2 (attention, per head, 4 t-strips of 512):
    scores^T [s,t] = kT-chunk.T @ qT-strip (K=64), exp on ACT with
    scale=1/8 folded in (scores bounded by |q||k|/8 <= 7.94 so max-
    subtraction-free softmax is fp32-safe), attention*V with lhsT =
    [v|1] accumulating oT [65, 512] over 16 s-chunks; row 64 = sum(exp).
    Division: DVE reciprocal on gathered [4,512] denominators, PE
    broadcast (ones[1,64].T @ recip row), DVE multiply, DMA out.
"""

import sys

if "/opt/trn_rl_repo" not in sys.path:
    sys.path.insert(0, "/opt/trn_rl_repo")

import numpy as np

B, D, T, H = 4, 1024, 2048, 16
NHL = 8            # heads per core
DH = 64            # head dim
NT = T // 128      # 16 t-tiles
ND = D // 128      # 8 d-chunks
NST = T // 512     # 4 t-strips

_prog = None


def _build(loop_n=None, part=None):
    import contextlib
    import concourse.bass as bass
    import concourse.bacc as bacc
    import concourse.tile as tile
    from concourse import mybir
    from concourse.masks import make_identity

    f32 = mybir.dt.float32
    f32r = mybir.dt.float32r
    AF = mybir.ActivationFunctionType
    ALU = mybir.AluOpType

    nc = bacc.Bacc()
    x_ext = nc.dram_tensor("x_local", [NT, ND, 128, 128], f32r, kind="ExternalInput")
    w_ext = nc.dram_tensor("w_local", [ND, 128, NHL * 192], f32r, kind="ExternalInput")
    out_ext = nc.dram_tensor("out_local", [NHL * DH, T], f32, kind="ExternalOutput")

    with tile.TileContext(nc) as tc:
      with (tc.For_i(0, loop_n, 1) if loop_n else contextlib.nullcontext()):
          with tc.tile_pool(name="persist", bufs=1) as persist, \
               tc.tile_pool(name="qkTp", bufs=1) as qkTp, \
               tc.tile_pool(name="vp", bufs=1) as vp:
              ident = persist.tile([128, 128], f32, tag="ident")
              make_identity(nc, ident)
              ones64f = persist.tile([1, 64], f32, tag="ones64f")
              nc.vector.memset(ones64f, 1.0)
              ones64 = persist.tile([1, 64], f32r, tag="ones64")
              nc.scalar.copy(ones64, ones64f)
              onecol = persist.tile([128, 1], f32, tag="onecol")
              nc.vector.memset(onecol, 1.0)

              # qTd[h]: [128, T] qhat^T duplicated on partitions 0:64 and
              # 64:128; kTi[h]: [128, T/2] khat^T with even s-chunks on
              # partitions 0:64, odd on 64:128. Lets two K=64 score matmuls
              # run concurrently in separate PE row-groups (tile_position).
              qTd = [qkTp.tile([128, T], f32r, tag=f"qTd{h}", name=f"qTd{h}")
                     for h in range(NHL)]
              kTi = [qkTp.tile([128, T // 2], f32r, tag=f"kTi{h}",
                               name=f"kTi{h}") for h in range(NHL)]
              # vt[h][i]: [s 128, 65] = [v | 1]
              vt = [[vp.tile([128, 65], f32r, tag=f"v{h}_{i}", name=f"v{h}_{i}")
                     for i in range(NT)] for h in range(NHL)]
              for h in range(NHL):
                  for i in range(NT):
                      nc.scalar.copy(vt[h][i][:, 64:65], onecol)

              if part == "p2":
                  for p in range(NHL):
                      nc.scalar.copy(qTd[p][:, 0:1], onecol)
                      nc.vector.memset(qTd[p].bitcast(f32), 0.01)
                      nc.vector.memset(kTi[p].bitcast(f32), 0.01)
                  for h in range(NHL):
                      for i in range(NT):
                          nc.vector.memset(vt[h][i][:, 0:64], 0.01)
              # ---------------- Phase 1: QKV projection + norm + transpose
              with tc.tile_pool(name="wsb", bufs=1) as wpool, \
                   tc.tile_pool(name="xin", bufs=2) as xpool, \
                   tc.tile_pool(name="stage", bufs=6) as stpool, \
                   tc.tile_pool(name="stats", bufs=8) as statp, \
                   tc.tile_pool(name="psumG", bufs=3, space="PSUM") as pgp, \
                   tc.tile_pool(name="trps", bufs=2, space="PSUM") as trp:
                  wsb = [wpool.tile([128, NHL * 192], f32r, tag=f"w{dc}", name=f"w{dc}")
                         for dc in range(ND)]
                  for dc in range(ND if part != "p2" else 0):
                      nc.sync.dma_start(out=wsb[dc], in_=w_ext[dc])

                  kstgs = {}
                  for ti in range(NT if part != "p2" else 0):
                      xts = []
                      for dc in range(ND):
                          xt = xpool.tile([128, 128], f32r, tag=f"x{dc}")
                          nc.sync.dma_start(out=xt, in_=x_ext[ti, dc])
                          xts.append(xt)
                      for half in range(2):
                          # psum [t128, 1024]: head-pair group g2 at col g2*512,
                          # cols g2*512 .. +384 used (within one 2KB bank)
                          ps = pgp.tile([128, 1024], f32, tag="pg")
                          for dc in range(ND):
                              for g2 in range(2):
                                  g = half * 2 + g2
                                  nc.tensor.matmul(
                                      ps[:, g2 * 512: g2 * 512 + 384],
                                      lhsT=xts[dc],
                                      rhs=wsb[dc][:, g * 384: (g + 1) * 384],
                                      start=(dc == 0),
                                      stop=(dc == ND - 1),
                                  )
                          for pr in range(2):
                              for m in range(2):
                                  h = (half * 2 + pr) * 2 + m
                                  base = pr * 512 + m * 192
                                  qstg = stpool.tile([128, 128], f32,
                                                     tag="qstg", name="qstg")
                                  if ti % 2 == 0:
                                      kstg = stpool.tile(
                                          [128, 128], f32, tag=f"kstg{h}",
                                          name=f"kstg{h}", bufs=2)
                                      kstgs[h] = kstg
                                  else:
                                      kstg = kstgs[h]
                                  # evacuate psum: q|k then v
                                  stg = stpool.tile([128, 128], f32, tag="stg")
                                  nc.scalar.copy(stg, ps[:, base: base + 128])
                                  nc.scalar.copy(
                                      vt[h][ti][:, 0:64],
                                      ps[:, base + 128: base + 192])
                                  # stats for q (group 0) and k (group 1)
                                  st6 = statp.tile([128, 2, 6], f32, tag="st6")
                                  nc.vector.bn_stats(st6[:, 0], stg[:, 0:64])
                                  nc.vector.bn_stats(st6[:, 1], stg[:, 64:128])
                                  mv = statp.tile([128, 4], f32, tag="mv")
                                  nc.vector.bn_aggr(mv[:, 0:2], st6[:, 0])
                                  nc.vector.bn_aggr(mv[:, 2:4], st6[:, 1])
                                  # sigma = sqrt(var*64/63); inv = 1/(sigma+1e-5)
                                  mv3 = mv.rearrange("p (a two) -> p a two", two=2)
                                  nc.scalar.activation(
                                      mv3[:, :, 1:2], mv3[:, :, 1:2], AF.Sqrt,
                                      scale=float(64.0 / 63.0))
                                  nc.vector.tensor_scalar_add(
                                      mv3[:, :, 1:2], mv3[:, :, 1:2], 1e-5)
                                  nc.vector.reciprocal(
                                      mv3[:, :, 1:2], mv3[:, :, 1:2])
                                  # qhat duplicated into both col halves
                                  nc.vector.tensor_scalar(
                                      out=qstg[:, 0:64],
                                      in0=stg[:, 0:64],
                                      scalar1=mv[:, 0:1], scalar2=mv[:, 1:2],
                                      op0=ALU.subtract, op1=ALU.mult)
                                  nc.vector.tensor_scalar(
                                      out=qstg[:, 64:128],
                                      in0=stg[:, 0:64],
                                      scalar1=mv[:, 0:1], scalar2=mv[:, 1:2],
                                      op0=ALU.subtract, op1=ALU.mult)
                                  # khat into the (ti%2) col half
                                  nc.vector.tensor_scalar(
                                      out=kstg[:, (ti % 2) * 64:
                                               (ti % 2) * 64 + 64],
                                      in0=stg[:, 64:128],
                                      scalar1=mv[:, 2:3], scalar2=mv[:, 3:4],
                                      op0=ALU.subtract, op1=ALU.mult)
                                  trq = trp.tile([128, 128], f32, tag="tr",
                                                 name="trq")
                                  nc.tensor.transpose(trq, qstg, ident)
                                  nc.vector.tensor_copy(
                                      qTd[h][:, ti * 128: (ti + 1) * 128], trq)
                                  if ti % 2 == 1:
                                      trk = trp.tile([128, 128], f32,
                                                     tag="tr", name="trk")
                                      nc.tensor.transpose(trk, kstg, ident)
                                      nc.scalar.copy(
                                          kTi[h][:, (ti // 2) * 128:
                                                 (ti // 2) * 128 + 128], trk)

              if part == "p1":
                  for p in range(NHL // 2):
                      nc.sync.dma_start(out=out_ext[p * 128:(p + 1) * 128, :],
                                        in_=qTd[p].bitcast(f32))
              # ---------------- Phase 2: attention per head
              with tc.tile_pool(name="pt", bufs=4) as ptp, \
                   tc.tile_pool(name="osb", bufs=6) as osbp, \
                   tc.tile_pool(name="outsb", bufs=4) as outp, \
                   tc.tile_pool(name="dt", bufs=2) as dtp, \
                   tc.tile_pool(name="reps", bufs=1, space="PSUM") as repp, \
                   tc.tile_pool(name="spsum", bufs=3, space="PSUM") as spp, \
                   tc.tile_pool(name="opsum", bufs=1, space="PSUM") as opp:
                  for h in range(NHL if part != "p1" else 0):
                      dt = dtp.tile([4, 512], f32, tag="dt")
                      rt = dtp.tile([4, 512], f32, tag="rt")
                      rtf = dtp.tile([1, 4 * 512], f32r, tag="rtf")
                      osbs = []
                      for st in range(NST):
                          op_ps = opp.tile([65, 512], f32, tag="op")
                          NJ = 8
                          LOOKAHEAD = 2
                          sps, pts = [], []

                          def emit_scores(j):
                              sp = spp.tile([128, 1024], f32, tag="sp",
                                            name=f"sp{j}")
                              for u in range(2):
                                  hb = u * 64
                                  nc.tensor.matmul(
                                      sp[:, u * 512: (u + 1) * 512],
                                      lhsT=kTi[h][hb: hb + 64,
                                                  j * 128: (j + 1) * 128],
                                      rhs=qTd[h][hb: hb + 64,
                                                 st * 512: (st + 1) * 512],
                                      start=True, stop=True,
                                      tile_position=(hb, 0))
                              pt = ptp.tile([128, 1024], f32r, tag="pt",
                                            name=f"pt{j}")
                              if part != "noexp":
                                  nc.scalar.activation(pt, sp, AF.Exp, scale=0.125)
                              else:
                                  nc.vector.memset(pt[:, 0:1], 1.0)
                              sps.append(sp)
                              pts.append(pt)

                          def emit_pv(j):
                              if part == "nopv":
                                  if j == 0:
                                      nc.vector.memset(op_ps[:, 0:1], 1.0)
                                  return
                              for u in range(2):
                                  sc = 2 * j + u
                                  nc.tensor.matmul(
                                      op_ps, lhsT=vt[h][sc],
                                      rhs=pts[j][:, u * 512: (u + 1) * 512],
                                      start=(sc == 0), stop=(sc == 15))

                          for j in range(LOOKAHEAD):
                              emit_scores(j)
                          for j in range(NJ):
                              if j + LOOKAHEAD < NJ:
                                  emit_scores(j + LOOKAHEAD)
                              emit_pv(j)
                          osb = osbp.tile([65, 512], f32, tag="osb")
                          nc.vector.tensor_copy(osb, op_ps)
                          nc.sync.dma_start(out=dt[st: st + 1, :], in_=osb[64:65, :])
                          osbs.append(osb)
                      nc.vector.reciprocal(rt, dt)
                      # flatten [4,512] partitions into partition 0's free
                      # dim; gpsimd DMA casts f32 -> f32r
                      nc.gpsimd.dma_start(out=rtf[0:1, :], in_=rt)
                      for st in range(NST):
                          rep = repp.tile([64, 512], f32, tag="rep",
                                          space="PSUM")
                          nc.tensor.matmul(
                              rep, lhsT=ones64,
                              rhs=rtf[0:1, st * 512: (st + 1) * 512],
                              start=True, stop=True)
                          outt = outp.tile([64, 512], f32, tag="outt")
                          nc.vector.tensor_mul(outt, osbs[st][0:64, :], rep)
                          nc.sync.dma_start(
                              out=out_ext[h * 64: (h + 1) * 64,
                                          st * 512: (st + 1) * 512],
                              in_=outt)
    nc.finalize()
    return nc


def _get_prog():
    global _prog
    if _prog is None:
        _prog = _build()
    return _prog


def make_in_maps(x, qkv):
    x = np.ascontiguousarray(np.asarray(x, dtype=np.float32))
    qkv = np.ascontiguousarray(np.asarray(qkv, dtype=np.float32))
    in_maps = []
    for c in range(8):
        b = c // 2
        hs = slice((c % 2) * 8, (c % 2) * 8 + 8)
        # [16 ti, 8 dc, 128 dp, 128 tf]
        xp = x[b].reshape(ND, 128, NT, 128).transpose(2, 0, 1, 3).copy()
        # [8 dc, 128 dp, h*192 + n*64 + e]
        wp = (qkv[:, hs].transpose(2, 1, 0, 3)
              .reshape(D, NHL * 192).reshape(ND, 128, NHL * 192).copy())
        in_maps.append({"x_local": xp, "w_local": wp})
    return in_maps


def gather(results):
    out = np.empty((B, D, T), np.float32)
    for c in range(8):
        out[c // 2, (c % 2) * 512: (c % 2) * 512 + 512, :] = \
            results[c]["out_local"]
    return out


def kernel(**inputs):
    from concourse.bass_utils import run_bass_kernel_spmd

    nc = _get_prog()
    in_maps = make_in_maps(inputs["x"], inputs["qkv"])
    res = run_bass_kernel_spmd(nc, in_maps, list(range(8)))
    return gather(res.results)

